# revision 3
# baseline (speedup 1.0000x reference)
"""Causal single-head attention on 8 Trainium2 NeuronCores.

Problem: x [32, 2048, 384] f32, Wq/Wk/Wv [384, 64] f32.
  q/k/v = x @ W;  out = softmax_causal(q k^T / sqrt(64)) @ v   -> [32, 2048, 64]

Strategy: data-parallel over batch (4 batches per core), weights replicated.
Per batch on one core (T=2048, C=384, H=64):
  - host pre-transposes x to xT [C, T] and casts inputs to bf16
  - qT/kT [H, T] via matmul with packed [Wq|Wk] stationaries
  - vT [H, T] -> PE-transposed to v tiles [s,H], augmented with a ones
    column (row 64 of the AV output then accumulates the softmax
    denominator Z for free)
  - scoresT [s, t] = kT_chunk^T-stationary x qT-moving; causal: only
    t >= s block-columns are computed
  - exp on ScalarE straight out of PSUM (scale=1/8 fused), bf16 to SBUF
  - AV: outT[65, t] accumulated over s-chunks in PSUM (v-aug stationary,
    expT moving)
  - PE-transpose outT -> [t, 65], reciprocal of col 64, per-partition
    scale, DMA out f32
No collectives needed.
"""

import sys

sys.path.insert(0, "/opt/trn_rl_repo")

import numpy as np
import ml_dtypes

import concourse.bass as bass
import concourse.mybir as mybir
import concourse.tile as tile
from concourse import bacc
from concourse.bass_utils import run_bass_kernel_spmd

BF16 = mybir.dt.bfloat16
F32 = mybir.dt.float32
NP_BF16 = ml_dtypes.bfloat16

B, T_FULL, C, H = 32, 2048, 384, 64
N_CORES = 8
B_LOC = B // N_CORES
SCALE = float(H) ** -0.5
Exp = mybir.ActivationFunctionType.Exp


def build_nc(b_loc=B_LOC, t=T_FULL):
    """Build the per-core Bass program (SPMD: same program on all cores)."""
    assert t % 512 == 0
    nc = bacc.Bacc(None, target_bir_lowering=False)
    cc = C // 128          # contraction chunks for projections
    ns = t // 128          # number of 128-wide s-chunks
    nt = t // 512          # number of 512-wide t-chunks

    xT = nc.declare_dram_parameter("xT", [b_loc, C, t], BF16, isOutput=False)
    wq_d = nc.declare_dram_parameter("Wq", [C, H], BF16, isOutput=False)
    wk_d = nc.declare_dram_parameter("Wk", [C, H], BF16, isOutput=False)
    wv_d = nc.declare_dram_parameter("Wv", [C, H], BF16, isOutput=False)
    id16_d = nc.declare_dram_parameter("ident16", [128, 128], BF16, isOutput=False)
    id32_d = nc.declare_dram_parameter("ident32", [128, 128], F32, isOutput=False)
    mask_d = nc.declare_dram_parameter("mask", [128, 128], BF16, isOutput=False)
    outp = nc.declare_dram_parameter("out", [b_loc, t, H], F32, isOutput=True)

    with tile.TileContext(nc) as tc:
        with (
            tc.tile_pool(name="consts", bufs=1) as consts,
            tc.tile_pool(name="xt", bufs=6) as p_xt,
            tc.tile_pool(name="qk", bufs=2) as p_qk,
            tc.tile_pool(name="vv", bufs=2) as p_v,
            tc.tile_pool(name="exp", bufs=3) as p_exp,
            tc.tile_pool(name="oo", bufs=2) as p_o,
            tc.tile_pool(name="ps_big", bufs=2, space="PSUM") as ps_big,
            tc.tile_pool(name="ps_out", bufs=4, space="PSUM") as ps_out,
        ):
            # ---- constants ----
            ident16 = consts.tile([128, 128], BF16)
            nc.sync.dma_start(out=ident16, in_=id16_d[:, :])
            ident32 = consts.tile([128, 128], F32)
            nc.sync.dma_start(out=ident32, in_=id32_d[:, :])
            dmask = consts.tile([128, 128], BF16)
            nc.sync.dma_start(out=dmask, in_=mask_d[:, :])
            # packed projection stationaries: [Wq|Wk] and [Wk|Wq] per c-chunk
            wqk = consts.tile([128, cc, 128], BF16)
            wkq = consts.tile([128, cc, 128], BF16)
            wv = consts.tile([128, cc, H], BF16)
            for c in range(cc):
                nc.sync.dma_start(out=wqk[:, c, 0:H], in_=wq_d[128 * c:128 * (c + 1), :])
                nc.sync.dma_start(out=wqk[:, c, H:128], in_=wk_d[128 * c:128 * (c + 1), :])
                nc.sync.dma_start(out=wkq[:, c, 0:H], in_=wk_d[128 * c:128 * (c + 1), :])
                nc.sync.dma_start(out=wkq[:, c, H:128], in_=wq_d[128 * c:128 * (c + 1), :])
                nc.sync.dma_start(out=wv[:, c, :], in_=wv_d[128 * c:128 * (c + 1), :])

            for b in range(b_loc):
                # ---- load xT (bf16, [C,t] per batch) ----
                xts = []
                for c in range(cc):
                    xt_sb = p_xt.tile([128, t], BF16, tag="xt")
                    nc.sync.dma_start(
                        out=xt_sb, in_=xT[b, 128 * c:128 * (c + 1), :]
                    )
                    xts.append(xt_sb)

                # ---- P1: projections ----
                # qT/kT [H, t] bf16 (rows 0..63). Computed via the packed
                # stationaries; rows 64..127 of the PSUM result are the other
                # projection, kept for a future row-packed scores variant.
                qT = p_qk.tile([128, t], BF16, tag="qT")
                kT = p_qk.tile([128, t], BF16, tag="kT")
                for tj in range(nt):
                    tr = slice(512 * tj, 512 * (tj + 1))
                    ps_g1 = ps_big.tile([128, 1024], F32, tag="big")
                    for c in range(cc):
                        nc.tensor.matmul(
                            ps_g1[:, 0:512], wqk[:, c, :], xts[c][:, tr],
                            start=(c == 0), stop=(c == cc - 1),
                        )
                    nc.vector.tensor_copy(qT[0:H, tr], ps_g1[0:H, 0:512])
                    ps_g2 = ps_big.tile([128, 1024], F32, tag="big")
                    for c in range(cc):
                        nc.tensor.matmul(
                            ps_g2[:, 0:512], wkq[:, c, :], xts[c][:, tr],
                            start=(c == 0), stop=(c == cc - 1),
                        )
                    nc.vector.tensor_copy(kT[0:H, tr], ps_g2[0:H, 0:512])

                # vT [H, t] then transpose into v-augmented tiles [128, ns, 65]
                vT = p_v.tile([64, t], BF16, tag="vT")
                for tj in range(nt):
                    tr = slice(512 * tj, 512 * (tj + 1))
                    ps_v = ps_big.tile([128, 1024], F32, tag="big")
                    for c in range(cc):
                        nc.tensor.matmul(
                            ps_v[0:H, 0:512], wv[:, c, :], xts[c][:, tr],
                            start=(c == 0), stop=(c == cc - 1),
                        )
                    nc.vector.tensor_copy(vT[:, tr], ps_v[0:H, 0:512])
                vaug = p_v.tile([128, ns, 65], BF16, tag="vaug")
                nc.vector.memset(vaug, 1.0)  # col 64 stays 1.0 (ones column)
                for st in range(ns):
                    ps_tr = ps_big.tile([128, 64], BF16, tag="big")
                    nc.tensor.transpose(
                        ps_tr, vT[:, 128 * st:128 * (st + 1)], ident16[0:H, 0:H]
                    )
                    nc.vector.tensor_copy(vaug[:, st, 0:H], ps_tr)

                # ---- P2: scores -> exp -> AV accumulate ----
                outTs = [None] * nt
                for i in range(ns):
                    jd = i // 4           # first valid 512-wide t-chunk
                    vstart = 128 * i      # first valid t column
                    expT = p_exp.tile([128, t], BF16, tag="expT")
                    if vstart > 512 * jd:
                        # sliver of the first AV strip below the diagonal
                        nc.vector.memset(expT[:, 512 * jd:vstart], 0.0)
                    # score strips of <=1024 psum columns
                    pos = vstart
                    while pos < t:
                        strip_lo = 512 * jd + ((pos - 512 * jd) // 1024) * 1024
                        strip_hi = min(strip_lo + 1024, t)
                        ps_s = ps_big.tile([128, 1024], F32, tag="big")
                        mmpos = pos
                        while mmpos < strip_hi:
                            seg_hi = min(mmpos + 512 - (mmpos % 512), strip_hi)
                            nc.tensor.matmul(
                                ps_s[:, mmpos - strip_lo:seg_hi - strip_lo],
                                kT[0:H, 128 * i:128 * (i + 1)],
                                qT[0:H, mmpos:seg_hi],
                                start=True, stop=True,
                            )
                            mmpos = seg_hi
                        nc.scalar.activation(
                            expT[:, pos:strip_hi],
                            ps_s[:, pos - strip_lo:strip_hi - strip_lo],
                            Exp, scale=SCALE,
                        )
                        pos = strip_hi
                    # causal mask on the diagonal 128x128 block
                    nc.vector.tensor_mul(
                        expT[:, vstart:vstart + 128],
                        expT[:, vstart:vstart + 128],
                        dmask,
                    )
                    # AV accumulate over s-chunks
                    for j in range(jd, nt):
                        if i == 0:
                            outTs[j] = ps_out.tile(
                                [65, 512], F32, tag="outT", name=f"outT{j}"
                            )
                        nc.tensor.matmul(
                            outTs[j],
                            vaug[:, i, :],
                            expT[:, 512 * j:512 * (j + 1)],
                            start=(i == 0), stop=(i == 4 * j + 3),
                        )
                        # ---- P3: finalize t-chunk j once complete ----
                        if i == 4 * j + 3:
                            outTn = p_o.tile([65, 512], F32, tag="outTn")
                            nc.vector.tensor_copy(outTn, outTs[j])
                            for tt in range(4):
                                ps_o = ps_out.tile([128, 65], F32, tag="outT")
                                nc.tensor.transpose(
                                    ps_o,
                                    outTn[:, 128 * tt:128 * (tt + 1)],
                                    ident32[0:65, 0:65],
                                )
                                zrec = p_o.tile([128, 1], F32, tag="zrec", bufs=4)
                                nc.vector.reciprocal(zrec, ps_o[:, H:H + 1])
                                o_sb = p_o.tile([128, H], F32, tag="o_sb", bufs=4)
                                nc.vector.tensor_scalar_mul(o_sb, ps_o[:, 0:H], zrec)
                                t0 = 512 * j + 128 * tt
                                nc.sync.dma_start(
                                    out=outp[b, t0:t0 + 128, :], in_=o_sb
                                )
    nc.compile()
    return nc


def _shard_inputs(x, Wk, Wq, Wv, b_loc=B_LOC, t=T_FULL):
    ident32 = np.eye(128, dtype=np.float32)
    ident16 = ident32.astype(NP_BF16)
    mask = np.triu(np.ones((128, 128), dtype=np.float32)).astype(NP_BF16)
    wq16 = np.ascontiguousarray(Wq, dtype=np.float32).astype(NP_BF16)
    wk16 = np.ascontiguousarray(Wk, dtype=np.float32).astype(NP_BF16)
    wv16 = np.ascontiguousarray(Wv, dtype=np.float32).astype(NP_BF16)
    n_cores = x.shape[0] // b_loc
    xs = np.asarray(x, dtype=np.float32).reshape(n_cores, b_loc, t, C)
    in_maps = []
    for m in range(n_cores):
        xT = np.ascontiguousarray(xs[m].transpose(0, 2, 1)).astype(NP_BF16)
        in_maps.append({
            "xT": xT, "Wq": wq16, "Wk": wk16, "Wv": wv16,
            "ident16": ident16, "ident32": ident32, "mask": mask,
        })
    return in_maps


def _run(x, Wk, Wq, Wv, trace=False, **spmd_kwargs):
    nc = build_nc()
    in_maps = _shard_inputs(x, Wk, Wq, Wv)
    res = run_bass_kernel_spmd(
        nc, in_maps, core_ids=list(range(N_CORES)), trace=trace, **spmd_kwargs
    )
    out = np.concatenate([res.results[m]["out"] for m in range(N_CORES)], axis=0)
    return np.ascontiguousarray(out, dtype=np.float32), res


def kernel(x, Wk, Wq, Wv):
    out, _ = _run(x, Wk, Wq, Wv)
    return out
